# revision 18
# baseline (speedup 1.0000x reference)
"""Causal GQA self-attention block (B=4, T=2048, C=1024, H=16, G=4) on 8
Trainium2 NeuronCores.

Sharding: core c = 2*b + h  (b in {0..3} batch-DP, h in {0,1} head-TP).
Each core handles batch b, kv groups {2h, 2h+1}, and the 8 heads that use
those groups (head j uses group j%4); it produces a bf16 partial projection
output and the host sums the 2 TP partials per batch and adds the bias.

Per-core kernel (all matmuls bf16, fp32 PSUM accumulation):
  - fused QKV projection from pre-transposed x (host supplies x^T in
    contiguous [cc, tile] blocks so DMA pipelines with the first matmuls),
    producing Q^T / K^T / V^T with channels on partitions
  - scores computed transposed (S^T[tk,tq] = K Q^T) in 128x512 tiles,
    head-pair packed into the PE array via tile_position (contraction=64)
  - causal: block skip + column trim + multiplicative triangular mask
  - unnormalized softmax: exp on ACT (scale folded), denominator obtained
    by appending a ones-column to V in the P@V matmul (M=65)
  - normalize via DVE reciprocal + gpsimd partition-broadcast + DVE mult
  - output projection on-device (bf16 partials); host sums TP partials
  - a short burst of dummy matmuls at t=0 keeps the PE busy during the
    initial x^T DMA so the HAM clock-gate releases before real work starts
"""

import os
import sys

sys.path.insert(0, "/opt/trn_rl_repo")

import numpy as np
from contextlib import ExitStack

import concourse.bass as bass
import concourse.mybir as mybir
import concourse.tile as tile
from concourse import bacc
from concourse.bass_utils import run_bass_kernel_spmd

# problem shape (hardcoded per contract)
B, T, C = 4, 2048, 1024
H, G = 16, 4
D = C // H  # 64

# per-core
NPAIR = 4        # head pairs per core (8 heads)
NGRP = 2         # kv groups per core
P = 128
CC = C // P      # 8 contraction chunks for projections
NT = 512         # tq tile width
TQT = T // NT    # 4 tq tiles
TKC = T // P     # 16 tk chunks

F32 = mybir.dt.float32
BF16 = mybir.dt.bfloat16
ADT = BF16
Exp = mybir.ActivationFunctionType.Exp
MULT = mybir.AluOpType.mult


def _build_program():
    nc = bacc.Bacc(None, target_bir_lowering=False)

    # x^T as [cc, partition, token] so merged bulk DMAs walk partition-major
    # (via a permuted access pattern) with 1KB contiguous runs
    xtl = nc.dram_tensor("xtl", [CC, P, T], ADT, kind="ExternalInput")
    # columns: q pair0..3 (4x128) | kv g0 (64 k + 64 v) | kv g1
    wqkv = nc.dram_tensor("wqkv", [P, CC, 768], ADT, kind="ExternalInput")
    wproj = nc.dram_tensor("wproj", [P, 4, C], ADT, kind="ExternalInput")
    # multiplicative triangular mask, duplicated for the 2 packed heads
    maskb = nc.dram_tensor("maskb", [P, 2, P], ADT, kind="ExternalInput")
    ident2 = nc.dram_tensor("ident2", [P, 64], ADT, kind="ExternalInput")
    vones = nc.dram_tensor("vones", [P, TKC], ADT, kind="ExternalInput")
    outp = nc.dram_tensor("outp", [T, C], ADT, kind="ExternalOutput")

    with tile.TileContext(nc) as tc:
        with ExitStack() as ctx:
            const = ctx.enter_context(tc.tile_pool(name="const", bufs=1))
            sb = ctx.enter_context(tc.tile_pool(name="sb", bufs=1))
            small = ctx.enter_context(tc.tile_pool(name="small", bufs=4))
            ppool = ctx.enter_context(tc.tile_pool(name="ppool", bufs=4))
            stg = ctx.enter_context(tc.tile_pool(name="stg", bufs=3))
            ps_st = ctx.enter_context(tc.tile_pool(name="ps_st", bufs=2, space="PSUM"))
            ps_pv = ctx.enter_context(tc.tile_pool(name="ps_pv", bufs=2, space="PSUM"))
            ps_mm = ctx.enter_context(tc.tile_pool(name="ps_mm", bufs=2, space="PSUM"))

            # ---- SBUF state ----
            wqkv_t = const.tile([P, CC, 768], ADT, tag="wqkv")
            wproj_t = const.tile([P, 4, C], ADT, tag="wproj")
            mask_t = const.tile([P, 2, P], ADT, tag="maskb")
            id2_t = const.tile([P, 64], ADT, tag="ident2")
            warm_t = const.tile([P, NT], ADT, tag="warm")
            xt = sb.tile([P, CC, T], ADT, tag="xt")
            q_sb = sb.tile([P, NPAIR, T], ADT, tag="q")
            # kv_sb rows 0:64 = K^T (kv-group), rows 64:128 = V^T
            kv_sb = sb.tile([P, NGRP, TQT, NT], ADT, tag="kv")
            k_hi = sb.tile([P, NGRP, TQT, NT], ADT, tag="khi")  # K dup rows 64:128
            v_a = sb.tile([P, NGRP, TKC, 65], ADT, tag="va")
            o_t = sb.tile([P, NPAIR, T], ADT, tag="ot")

            # ---- DMA plan ----
            # All bulk loads are merged into a handful of big up-front DMA
            # instructions (each ~600ns of queue issue time, so fewer is
            # better) and kept OFF the sync queue: a dependent DMA (k dup,
            # denom shift, o_t shift, output store) at the head of a queue
            # blocks everything behind it, so sync carries only those small
            # latency-critical transfers.
            def permute_ap(ap, order):
                return bass.AP(ap.tensor, ap.offset, [ap.ap[i] for i in order])

            def xt_load(eng, clo, chi, tlo, thi):
                eng.dma_start(
                    xt[:, clo:chi, tlo * NT : thi * NT],
                    permute_ap(xtl[clo:chi, :, tlo * NT : thi * NT], [1, 0, 2]),
                )

            # single bulk queue: per-queue descriptor processing is roughly
            # in-order, so queue position = transfer priority (weights and
            # tile-0 first so the first QKV chains start ~6us in)
            nc.gpsimd.dma_start(wqkv_t[:, 0:4, :], wqkv[:, 0:4, :])
            nc.gpsimd.dma_start(wqkv_t[:, 4:8, :], wqkv[:, 4:8, :])
            xt_load(nc.gpsimd, 0, 4, 0, 1)
            xt_load(nc.gpsimd, 4, 8, 0, 1)
            xt_load(nc.gpsimd, 0, 4, 1, 2)
            xt_load(nc.gpsimd, 4, 8, 1, 2)
            xt_load(nc.gpsimd, 0, 4, 2, 4)
            xt_load(nc.gpsimd, 4, 8, 2, 4)
            nc.gpsimd.dma_start(wproj_t[:], wproj[:])
            nc.scalar.dma_start(mask_t[:], maskb[:])
            nc.scalar.dma_start(id2_t[:], ident2[:])
            for g in range(NGRP):
                nc.scalar.dma_start(v_a[:, g, :, 64], vones[:])

            # ---- PE warmup: keep the array busy during the initial DMA so
            # the HAM clock-gate goes 8/8 before real work ----
            nc.vector.memset(warm_t[:], 0.0)

            def emit_warm(k):
                for w in range(k):
                    pmw = ps_mm.tile([P, NT], F32, tag="mm")
                    nc.tensor.matmul(
                        pmw[:], warm_t[:, 0:P], warm_t[:], start=True, stop=True
                    )

            emit_warm(12)

            def emit_qkv_part(n, m):
                # ---- QKV projection tile n, part m (0..3: q pair m,
                # 4/5: kv group m-4 incl. V transpose + k dup) ----
                pm = ps_mm.tile([P, NT], F32, tag="mm")
                for cc in range(CC):
                    nc.tensor.matmul(
                        pm[:],
                        wqkv_t[:, cc, m * P : (m + 1) * P],
                        xt[:, cc, n * NT : (n + 1) * NT],
                        start=(cc == 0),
                        stop=(cc == CC - 1),
                    )
                if m < 4:
                    nc.vector.tensor_copy(q_sb[:, m, n * NT : (n + 1) * NT], pm[:])
                    return
                g = m - 4
                nc.vector.tensor_copy(kv_sb[:, g, n, :], pm[:])
                nc.sync.dma_start(k_hi[64:128, g, n, :], kv_sb[0:64, g, n, :])
                for i in range(4 * n, 4 * n + 4):
                    pt = ps_mm.tile([P, 64], ADT, tag="mm")
                    nc.tensor.transpose(
                        pt[:],
                        kv_sb[64:128, g, i // 4, (i % 4) * P : (i % 4 + 1) * P],
                        id2_t[64:128, :],
                    )
                    nc.vector.tensor_copy(v_a[:, g, i, 0:64], pt[:])

            def emit_attn_jp(j, p_, tail=False):
                g = p_ // 2
                pv = [
                    ps_pv.tile([P, NT], F32, tag="pv", name=f"pv{e}")
                    for e in range(2)
                ]
                last = 4 * j + 3
                for i in range(4 * j + 4):
                    diag = i >= 4 * j
                    r = i - 4 * j
                    lo = r * P if diag else 0
                    st = ps_st.tile([P, 2, NT], F32, tag="st")
                    for e in range(2):
                        ksrc = kv_sb if e == 0 else k_hi
                        nc.tensor.matmul(
                            st[:, e, lo:NT],
                            ksrc[
                                64 * e : 64 * e + 64,
                                g,
                                i // 4,
                                (i % 4) * P : (i % 4 + 1) * P,
                            ],
                            q_sb[
                                64 * e : 64 * e + 64,
                                p_,
                                j * NT + lo : (j + 1) * NT,
                            ],
                            start=True,
                            stop=True,
                            tile_position=(64 * e, 0),
                        )
                    pexp = ppool.tile([P, 2, NT], ADT, tag="pexp")
                    nc.scalar.activation(
                        pexp[:, :, lo:NT],
                        st[:, :, lo:NT],
                        Exp,
                        scale=0.125,
                    )
                    if diag:
                        nc.vector.tensor_tensor(
                            pexp[:, :, lo : lo + P],
                            pexp[:, :, lo : lo + P],
                            mask_t[:],
                            MULT,
                        )
                    for e in range(2):
                        nc.tensor.matmul(
                            pv[e][0:65, lo:NT],
                            v_a[:, g, i, :],
                            pexp[:, e, lo:NT],
                            start=(i == 0),
                            stop=(i == last),
                        )
                # normalize: o = pv[0:64] / pv[64].  Stage-pipelined across
                # the two heads (e=1 first: its chain is longer, ending in a
                # partition-shift DMA) so the DVE FIFO doesn't serialize the
                # full chain of one head before the other starts.  In the
                # last unit (tail) the ACT engine is idle, so it takes the
                # psum evacuations.
                pvs, l0, rec0, bca = {}, {}, {}, {}
                for e in (1, 0):
                    # copy psum out early to release the PV bank
                    pvs[e] = small.tile([65, NT], F32, tag=f"pvs{e}", name=f"pvs{e}")
                    if tail:
                        nc.scalar.copy(pvs[e][:], pv[e][0:65, :])
                    else:
                        nc.vector.tensor_copy(pvs[e][:], pv[e][0:65, :])
                for e in (1, 0):
                    # reciprocal_approx_fast and partition_broadcast both
                    # require absolute partition 0 on HW: shift the
                    # denominator row down first
                    l0[e] = small.tile([1, NT], F32, tag=f"l0{e}", name=f"l0{e}")
                    nc.sync.dma_start(l0[e][:], pvs[e][64:65, :])
                for e in (1, 0):
                    rec0[e] = small.tile([1, NT], F32, tag=f"rec0{e}", name=f"rec0{e}")
                    nc.vector.reciprocal_approx_fast(rec0[e][:], l0[e][:])
                for e in (1, 0):
                    bca[e] = small.tile([64, NT], F32, tag=f"bca{e}", name=f"bca{e}")
                    nc.gpsimd.partition_broadcast(bca[e][:], rec0[e][:])
                otmp = small.tile([64, NT], ADT, tag="otmp")
                nc.vector.tensor_tensor(otmp[:], pvs[1][0:64, :], bca[1][:], MULT)
                nc.sync.dma_start(
                    o_t[64:128, p_, j * NT : (j + 1) * NT], otmp[:]
                )
                nc.vector.tensor_tensor(
                    o_t[0:64, p_, j * NT : (j + 1) * NT],
                    pvs[0][0:64, :],
                    bca[0][:],
                    MULT,
                )

            def emit_proj_t(t_, tail=False):
                # ---- output projection for one tq chunk (bf16 partial) ----
                # tail units split the psum evacuation between ACT (idle by
                # then) and DVE, and alternate store queues, to shorten the
                # final drain
                stage = stg.tile([P, C], ADT, tag="stage")
                for n2 in range(2):
                    pm = ps_mm.tile([P, NT], F32, tag="mm")
                    for cc2 in range(4):
                        nc.tensor.matmul(
                            pm[:],
                            o_t[:, cc2, t_ * P : (t_ + 1) * P],
                            wproj_t[:, cc2, n2 * NT : (n2 + 1) * NT],
                            start=(cc2 == 0),
                            stop=(cc2 == 3),
                        )
                    if tail and n2 == 0:
                        nc.scalar.copy(stage[:, n2 * NT : (n2 + 1) * NT], pm[:])
                    else:
                        nc.vector.tensor_copy(
                            stage[:, n2 * NT : (n2 + 1) * NT], pm[:]
                        )
                    if tail:
                        eng = nc.scalar if n2 else nc.sync
                        eng.dma_start(
                            outp[t_ * P : (t_ + 1) * P, n2 * NT : (n2 + 1) * NT],
                            stage[:, n2 * NT : (n2 + 1) * NT],
                        )
                if not tail:
                    nc.sync.dma_start(outp[t_ * P : (t_ + 1) * P, :], stage[:])

            # fill plan: qkv parts, proj chunks and bulk DMAs threaded
            # between attention (j, pair) units so the ACT (exp) stream
            # never starves the PE.  Tile epochs run in order 0,1,3,2 so
            # tile-3's proj chunks are available to fill tile-2's attention
            # and only tile-2's proj trails the last unit.
            # part m: 0..3 q pairs, 4/5 kv groups; QK(n, m); PR(t);
            # XT(n, c2) loads x^T chunks 2*c2, 2*c2+1 of tile n; WP(k)
            # wproj chunk k; DW(k) = k dummy warm matmuls (HAM insurance
            # through the DMA-paced start)
            QK = lambda n, m: ("qkv", n, m)
            PR = lambda t: ("proj", t)
            DW = lambda k: ("warm", k)
            plan = {
                (0, 0): [QK(0, 1), QK(0, 5)],
                (0, 1): [QK(0, 2), QK(0, 3), DW(1)],
                (0, 2): [QK(1, 4), QK(1, 0), DW(1)],
                (0, 3): [QK(1, 1), QK(1, 5), DW(1)],
                (1, 0): [QK(1, 2), QK(1, 3), DW(1)],
                # kv parts of EVERY tile (incl. tile 2) must land before the
                # tile-3 epoch: its attention spans tk tiles 0..3
                (1, 1): [QK(2, 4), QK(2, 5)],
                (1, 2): [QK(3, 4), QK(3, 0)],
                (1, 3): [QK(3, 1), QK(3, 5)],
                (3, 0): [QK(3, 2), QK(3, 3), PR(0), PR(1)],
                (3, 1): [QK(2, 0), QK(2, 1), PR(2), PR(3)],
                (3, 2): [QK(2, 2), QK(2, 3), PR(4), PR(5)],
                (3, 3): [PR(6), PR(7)],
                (2, 0): [PR(12)],
                (2, 1): [PR(13)],
                (2, 2): [PR(14)],
                (2, 3): [PR(15)],
            }
            emit_qkv_part(0, 4)
            emit_qkv_part(0, 0)
            for j in (0, 1, 3, 2):
                for p_ in range(NPAIR):
                    emit_attn_jp(j, p_, tail=(j == 2 and p_ == 3))
                    for f in plan[(j, p_)]:
                        if f[0] == "qkv":
                            emit_qkv_part(f[1], f[2])
                        elif f[0] == "proj":
                            emit_proj_t(f[1])
                        elif f[0] == "warm":
                            emit_warm(f[1])
            for t_ in range(8, 12):
                emit_proj_t(t_, tail=True)

    nc.compile()
    return nc


_NC = None


def _get_program():
    global _NC
    if _NC is None:
        _NC = _build_program()
    return _NC


def _host_inputs(x, Wq, Wkv, Wproj):
    """Shard + lay out inputs for the 8 cores."""
    import ml_dtypes

    adt_np = ml_dtypes.bfloat16
    # multiplicative mask: keep where tk_local <= tq_local
    tri = np.where(
        np.arange(P)[:, None] <= np.arange(P)[None, :], 1.0, 0.0
    ).astype(np.float32)
    ident2 = np.concatenate([np.eye(64, dtype=np.float32)] * 2, axis=0).astype(
        adt_np
    )  # [128, 64]
    maskb = np.stack([tri, tri], axis=1).astype(adt_np)  # [128, 2, 128]
    vones = np.ones((P, TKC), dtype=adt_np)

    # per-TP-half weights (shared across the 4 batches)
    half_w = []
    for h in range(2):
        groups = [2 * h, 2 * h + 1]
        # pair p uses kv group p//2; order must match proj channel order below
        pairs = [(groups[0], groups[0] + 4), (groups[0] + 8, groups[0] + 12),
                 (groups[1], groups[1] + 4), (groups[1] + 8, groups[1] + 12)]
        cols = []
        for a, b in pairs:
            cols.append(Wq[a * D : (a + 1) * D, :].T)
            cols.append(Wq[b * D : (b + 1) * D, :].T)
        for g in groups:
            cols.append(Wkv[g * D : (g + 1) * D, :].T)  # k
            cols.append(Wkv[G * D + g * D : G * D + (g + 1) * D, :].T)  # v
        wqkv = np.concatenate(cols, axis=1).astype(adt_np)  # [1024, 768]
        wqkv4 = np.ascontiguousarray(wqkv.reshape(CC, P, 768).transpose(1, 0, 2))
        ch = np.concatenate(
            [np.arange(hh * D, (hh + 1) * D) for a, b in pairs for hh in (a, b)]
        )
        wproj_s = np.ascontiguousarray(Wproj[:, ch].T).astype(adt_np)  # [512,1024]
        wproj4 = np.ascontiguousarray(wproj_s.reshape(4, P, C).transpose(1, 0, 2))
        half_w.append((wqkv4, wproj4))

    in_maps = []
    for b in range(B):
        xT = np.ascontiguousarray(x[b].T).astype(adt_np)  # [1024, 2048]
        xt4 = np.ascontiguousarray(xT.reshape(CC, P, T))
        for h in range(2):
            wqkv4, wproj4 = half_w[h]
            in_maps.append(
                {
                    "xtl": xt4,
                    "wqkv": wqkv4,
                    "wproj": wproj4,
                    "maskb": maskb,
                    "ident2": ident2,
                    "vones": vones,
                }
            )
    return in_maps


def kernel(x, Wq, Wkv, Wproj, b_proj):
    x = np.asarray(x, dtype=np.float32)
    Wq = np.asarray(Wq, dtype=np.float32)
    Wkv = np.asarray(Wkv, dtype=np.float32)
    Wproj = np.asarray(Wproj, dtype=np.float32)
    b_proj = np.asarray(b_proj, dtype=np.float32)

    nc = _get_program()
    in_maps = _host_inputs(x, Wq, Wkv, Wproj)
    trace = bool(int(os.environ.get("BASS_KERNEL_TRACE", "0")))
    res = run_bass_kernel_spmd(nc, in_maps, list(range(8)), trace=trace)
    if trace:
        kernel.last_results = res

    out = np.empty((B, T, C), dtype=np.float32)
    for b in range(B):
        acc = res.results[2 * b]["outp"].astype(np.float32)
        acc = acc + res.results[2 * b + 1]["outp"].astype(np.float32)
        out[b] = acc + b_proj[None, :]
    return out


# revision 19
# speedup vs baseline: 1.1123x; 1.1123x over previous
"""Causal GQA self-attention block (B=4, T=2048, C=1024, H=16, G=4) on 8
Trainium2 NeuronCores.

Sharding: core c = 2*b + h  (b in {0..3} batch-DP, h in {0,1} head-TP).
Each core handles batch b, kv groups {2h, 2h+1}, and the 8 heads that use
those groups (head j uses group j%4); it produces a bf16 partial projection
output and the host sums the 2 TP partials per batch and adds the bias.

Per-core kernel (all matmuls bf16, fp32 PSUM accumulation):
  - fused QKV projection from pre-transposed x (host supplies x^T in
    contiguous [cc, tile] blocks so DMA pipelines with the first matmuls),
    producing Q^T / K^T / V^T with channels on partitions
  - scores computed transposed (S^T[tk,tq] = K Q^T) in 128x512 tiles,
    head-pair packed into the PE array via tile_position (contraction=64)
  - causal: block skip + column trim + multiplicative triangular mask
  - unnormalized softmax: exp on ACT (scale folded), denominator obtained
    by appending a ones-column to V in the P@V matmul (M=65)
  - normalize via DVE reciprocal + gpsimd partition-broadcast + DVE mult
  - output projection on-device (bf16 partials); host sums TP partials
  - a short burst of dummy matmuls at t=0 keeps the PE busy during the
    initial x^T DMA so the HAM clock-gate releases before real work starts
"""

import os
import sys

sys.path.insert(0, "/opt/trn_rl_repo")

import numpy as np
from contextlib import ExitStack

import concourse.bass as bass
import concourse.mybir as mybir
import concourse.tile as tile
from concourse import bacc
from concourse.bass_utils import run_bass_kernel_spmd

# problem shape (hardcoded per contract)
B, T, C = 4, 2048, 1024
H, G = 16, 4
D = C // H  # 64

# per-core
NPAIR = 4        # head pairs per core (8 heads)
NGRP = 2         # kv groups per core
P = 128
CC = C // P      # 8 contraction chunks for projections
NT = 512         # tq tile width
TQT = T // NT    # 4 tq tiles
TKC = T // P     # 16 tk chunks

F32 = mybir.dt.float32
BF16 = mybir.dt.bfloat16
ADT = BF16
Exp = mybir.ActivationFunctionType.Exp
MULT = mybir.AluOpType.mult


def _build_program():
    nc = bacc.Bacc(None, target_bir_lowering=False)

    # x^T as [cc, partition, token] so merged bulk DMAs walk partition-major
    # (via a permuted access pattern) with 1KB contiguous runs
    xtl = nc.dram_tensor("xtl", [CC, P, T], ADT, kind="ExternalInput")
    # columns: q pair0..3 (4x128) | kv g0 (64 k + 64 v) | kv g1
    wqkv = nc.dram_tensor("wqkv", [P, CC, 768], ADT, kind="ExternalInput")
    wproj = nc.dram_tensor("wproj", [P, 4, C], ADT, kind="ExternalInput")
    # multiplicative triangular mask, duplicated for the 2 packed heads
    maskb = nc.dram_tensor("maskb", [P, 2, P], ADT, kind="ExternalInput")
    ident2 = nc.dram_tensor("ident2", [P, 64], ADT, kind="ExternalInput")
    vones = nc.dram_tensor("vones", [P, TKC], ADT, kind="ExternalInput")
    outp = nc.dram_tensor("outp", [T, C], ADT, kind="ExternalOutput")

    with tile.TileContext(nc) as tc:
        with ExitStack() as ctx:
            const = ctx.enter_context(tc.tile_pool(name="const", bufs=1))
            sb = ctx.enter_context(tc.tile_pool(name="sb", bufs=1))
            small = ctx.enter_context(tc.tile_pool(name="small", bufs=4))
            ppool = ctx.enter_context(tc.tile_pool(name="ppool", bufs=4))
            stg = ctx.enter_context(tc.tile_pool(name="stg", bufs=3))
            ps_st = ctx.enter_context(tc.tile_pool(name="ps_st", bufs=2, space="PSUM"))
            ps_pv = ctx.enter_context(tc.tile_pool(name="ps_pv", bufs=2, space="PSUM"))
            ps_mm = ctx.enter_context(tc.tile_pool(name="ps_mm", bufs=2, space="PSUM"))

            # ---- SBUF state ----
            wqkv_t = const.tile([P, CC, 768], ADT, tag="wqkv")
            wproj_t = const.tile([P, 4, C], ADT, tag="wproj")
            mask_t = const.tile([P, 2, P], ADT, tag="maskb")
            id2_t = const.tile([P, 64], ADT, tag="ident2")
            warm_t = const.tile([P, NT], ADT, tag="warm")
            xt = sb.tile([P, CC, T], ADT, tag="xt")
            q_sb = sb.tile([P, NPAIR, T], ADT, tag="q")
            # kv_sb rows 0:64 = K^T (kv-group), rows 64:128 = V^T
            kv_sb = sb.tile([P, NGRP, TQT, NT], ADT, tag="kv")
            k_hi = sb.tile([P, NGRP, TQT, NT], ADT, tag="khi")  # K dup rows 64:128
            v_a = sb.tile([P, NGRP, TKC, 65], ADT, tag="va")
            o_t = sb.tile([P, NPAIR, T], ADT, tag="ot")

            # ---- DMA plan ----
            # All bulk loads are merged into a handful of big up-front DMA
            # instructions (each ~600ns of queue issue time, so fewer is
            # better) and kept OFF the sync queue: a dependent DMA (k dup,
            # denom shift, o_t shift, output store) at the head of a queue
            # blocks everything behind it, so sync carries only those small
            # latency-critical transfers.
            def permute_ap(ap, order):
                return bass.AP(ap.tensor, ap.offset, [ap.ap[i] for i in order])

            def xt_load(eng, clo, chi, tlo, thi):
                eng.dma_start(
                    xt[:, clo:chi, tlo * NT : thi * NT],
                    permute_ap(xtl[clo:chi, :, tlo * NT : thi * NT], [1, 0, 2]),
                )

            # DMA packets interleave across instructions even within one
            # queue, so queue order alone gives no transfer priority.  Load
            # only the first-needed 2.5MB (weights + x^T tile 0) up-front on
            # both queues; the remaining 4MB is emitted on sync BEHIND the
            # first k-dup DMA, which blocks until the tile-0 QKV chain
            # completes -- a hard serialization that keeps the late bulk from
            # contending with tile-0.
            nc.sync.dma_start(wqkv_t[:, 0:4, :], wqkv[:, 0:4, :])
            xt_load(nc.sync, 0, 4, 0, 1)
            nc.gpsimd.dma_start(wqkv_t[:, 4:8, :], wqkv[:, 4:8, :])
            xt_load(nc.gpsimd, 4, 8, 0, 1)
            nc.scalar.dma_start(mask_t[:], maskb[:])
            nc.scalar.dma_start(id2_t[:], ident2[:])
            for g in range(NGRP):
                nc.scalar.dma_start(v_a[:, g, :, 64], vones[:])

            # ---- PE warmup: keep the array busy during the initial DMA so
            # the HAM clock-gate goes 8/8 before real work ----
            nc.vector.memset(warm_t[:], 0.0)

            def emit_warm(k):
                for w in range(k):
                    pmw = ps_mm.tile([P, NT], F32, tag="mm")
                    nc.tensor.matmul(
                        pmw[:], warm_t[:, 0:P], warm_t[:], start=True, stop=True
                    )

            emit_warm(12)

            def emit_qkv_part(n, m):
                # ---- QKV projection tile n, part m (0..3: q pair m,
                # 4/5: kv group m-4 incl. V transpose + k dup) ----
                pm = ps_mm.tile([P, NT], F32, tag="mm")
                for cc in range(CC):
                    nc.tensor.matmul(
                        pm[:],
                        wqkv_t[:, cc, m * P : (m + 1) * P],
                        xt[:, cc, n * NT : (n + 1) * NT],
                        start=(cc == 0),
                        stop=(cc == CC - 1),
                    )
                if m < 4:
                    nc.vector.tensor_copy(q_sb[:, m, n * NT : (n + 1) * NT], pm[:])
                    return
                g = m - 4
                nc.vector.tensor_copy(kv_sb[:, g, n, :], pm[:])
                nc.sync.dma_start(k_hi[64:128, g, n, :], kv_sb[0:64, g, n, :])
                for i in range(4 * n, 4 * n + 4):
                    pt = ps_mm.tile([P, 64], ADT, tag="mm")
                    nc.tensor.transpose(
                        pt[:],
                        kv_sb[64:128, g, i // 4, (i % 4) * P : (i % 4 + 1) * P],
                        id2_t[64:128, :],
                    )
                    nc.vector.tensor_copy(v_a[:, g, i, 0:64], pt[:])

            def emit_attn_jp(j, p_, tail=False):
                g = p_ // 2
                pv = [
                    ps_pv.tile([P, NT], F32, tag="pv", name=f"pv{e}")
                    for e in range(2)
                ]
                last = 4 * j + 3
                for i in range(4 * j + 4):
                    diag = i >= 4 * j
                    r = i - 4 * j
                    lo = r * P if diag else 0
                    st = ps_st.tile([P, 2, NT], F32, tag="st")
                    for e in range(2):
                        ksrc = kv_sb if e == 0 else k_hi
                        nc.tensor.matmul(
                            st[:, e, lo:NT],
                            ksrc[
                                64 * e : 64 * e + 64,
                                g,
                                i // 4,
                                (i % 4) * P : (i % 4 + 1) * P,
                            ],
                            q_sb[
                                64 * e : 64 * e + 64,
                                p_,
                                j * NT + lo : (j + 1) * NT,
                            ],
                            start=True,
                            stop=True,
                            tile_position=(64 * e, 0),
                        )
                    pexp = ppool.tile([P, 2, NT], ADT, tag="pexp")
                    nc.scalar.activation(
                        pexp[:, :, lo:NT],
                        st[:, :, lo:NT],
                        Exp,
                        scale=0.125,
                    )
                    if diag:
                        nc.vector.tensor_tensor(
                            pexp[:, :, lo : lo + P],
                            pexp[:, :, lo : lo + P],
                            mask_t[:],
                            MULT,
                        )
                    for e in range(2):
                        nc.tensor.matmul(
                            pv[e][0:65, lo:NT],
                            v_a[:, g, i, :],
                            pexp[:, e, lo:NT],
                            start=(i == 0),
                            stop=(i == last),
                        )
                # normalize: o = pv[0:64] / pv[64].  Stage-pipelined across
                # the two heads (e=1 first: its chain is longer, ending in a
                # partition-shift DMA) so the DVE FIFO doesn't serialize the
                # full chain of one head before the other starts.  In the
                # last unit (tail) the ACT engine is idle, so it takes the
                # psum evacuations.
                pvs, l0, rec0, bca = {}, {}, {}, {}
                for e in (1, 0):
                    # copy psum out early to release the PV bank
                    pvs[e] = small.tile([65, NT], F32, tag=f"pvs{e}", name=f"pvs{e}")
                    if tail:
                        nc.scalar.copy(pvs[e][:], pv[e][0:65, :])
                    else:
                        nc.vector.tensor_copy(pvs[e][:], pv[e][0:65, :])
                for e in (1, 0):
                    # reciprocal_approx_fast and partition_broadcast both
                    # require absolute partition 0 on HW: shift the
                    # denominator row down first
                    l0[e] = small.tile([1, NT], F32, tag=f"l0{e}", name=f"l0{e}")
                    nc.sync.dma_start(l0[e][:], pvs[e][64:65, :])
                for e in (1, 0):
                    rec0[e] = small.tile([1, NT], F32, tag=f"rec0{e}", name=f"rec0{e}")
                    nc.vector.reciprocal_approx_fast(rec0[e][:], l0[e][:])
                for e in (1, 0):
                    bca[e] = small.tile([64, NT], F32, tag=f"bca{e}", name=f"bca{e}")
                    nc.gpsimd.partition_broadcast(bca[e][:], rec0[e][:])
                otmp = small.tile([64, NT], ADT, tag="otmp")
                nc.vector.tensor_tensor(otmp[:], pvs[1][0:64, :], bca[1][:], MULT)
                nc.sync.dma_start(
                    o_t[64:128, p_, j * NT : (j + 1) * NT], otmp[:]
                )
                nc.vector.tensor_tensor(
                    o_t[0:64, p_, j * NT : (j + 1) * NT],
                    pvs[0][0:64, :],
                    bca[0][:],
                    MULT,
                )

            def emit_proj_t(t_, tail=False):
                # ---- output projection for one tq chunk (bf16 partial) ----
                # tail units split the psum evacuation between ACT (idle by
                # then) and DVE, and alternate store queues, to shorten the
                # final drain
                stage = stg.tile([P, C], ADT, tag="stage")
                for n2 in range(2):
                    pm = ps_mm.tile([P, NT], F32, tag="mm")
                    for cc2 in range(4):
                        nc.tensor.matmul(
                            pm[:],
                            o_t[:, cc2, t_ * P : (t_ + 1) * P],
                            wproj_t[:, cc2, n2 * NT : (n2 + 1) * NT],
                            start=(cc2 == 0),
                            stop=(cc2 == 3),
                        )
                    if tail and n2 == 0:
                        nc.scalar.copy(stage[:, n2 * NT : (n2 + 1) * NT], pm[:])
                    else:
                        nc.vector.tensor_copy(
                            stage[:, n2 * NT : (n2 + 1) * NT], pm[:]
                        )
                    if tail:
                        eng = nc.scalar if n2 else nc.sync
                        eng.dma_start(
                            outp[t_ * P : (t_ + 1) * P, n2 * NT : (n2 + 1) * NT],
                            stage[:, n2 * NT : (n2 + 1) * NT],
                        )
                if not tail:
                    nc.sync.dma_start(outp[t_ * P : (t_ + 1) * P, :], stage[:])

            # fill plan: qkv parts, proj chunks and bulk DMAs threaded
            # between attention (j, pair) units so the ACT (exp) stream
            # never starves the PE.  Tile epochs run in order 0,1,3,2 so
            # tile-3's proj chunks are available to fill tile-2's attention
            # and only tile-2's proj trails the last unit.
            # part m: 0..3 q pairs, 4/5 kv groups; QK(n, m); PR(t);
            # XT(n, c2) loads x^T chunks 2*c2, 2*c2+1 of tile n; WP(k)
            # wproj chunk k; DW(k) = k dummy warm matmuls (HAM insurance
            # through the DMA-paced start)
            QK = lambda n, m: ("qkv", n, m)
            PR = lambda t: ("proj", t)
            DW = lambda k: ("warm", k)
            plan = {
                (0, 0): [QK(0, 1), QK(0, 5)],
                (0, 1): [QK(0, 2), QK(0, 3), DW(1)],
                (0, 2): [QK(1, 4), QK(1, 0), DW(1)],
                (0, 3): [QK(1, 1), QK(1, 5), DW(1)],
                (1, 0): [QK(1, 2), QK(1, 3), DW(1)],
                # kv parts of EVERY tile (incl. tile 2) must land before the
                # tile-3 epoch: its attention spans tk tiles 0..3
                (1, 1): [QK(2, 4), QK(2, 5)],
                (1, 2): [QK(3, 4), QK(3, 0)],
                (1, 3): [QK(3, 1), QK(3, 5)],
                (3, 0): [QK(3, 2), QK(3, 3), PR(0), PR(1)],
                (3, 1): [QK(2, 0), QK(2, 1), PR(2), PR(3)],
                (3, 2): [QK(2, 2), QK(2, 3), PR(4), PR(5)],
                (3, 3): [PR(6), PR(7)],
                (2, 0): [PR(12)],
                (2, 1): [PR(13)],
                (2, 2): [PR(14)],
                (2, 3): [PR(15)],
            }
            emit_qkv_part(0, 4)
            # late bulk: queued on sync behind the k-dup just emitted above
            xt_load(nc.sync, 0, 4, 1, 2)
            xt_load(nc.sync, 4, 8, 1, 2)
            xt_load(nc.sync, 0, 4, 2, 4)
            xt_load(nc.sync, 4, 8, 2, 4)
            nc.sync.dma_start(wproj_t[:], wproj[:])
            emit_qkv_part(0, 0)
            for j in (0, 1, 3, 2):
                for p_ in range(NPAIR):
                    emit_attn_jp(j, p_, tail=(j == 2 and p_ == 3))
                    for f in plan[(j, p_)]:
                        if f[0] == "qkv":
                            emit_qkv_part(f[1], f[2])
                        elif f[0] == "proj":
                            emit_proj_t(f[1])
                        elif f[0] == "warm":
                            emit_warm(f[1])
            for t_ in range(8, 12):
                emit_proj_t(t_, tail=True)

    nc.compile()
    return nc


_NC = None


def _get_program():
    global _NC
    if _NC is None:
        _NC = _build_program()
    return _NC


def _host_inputs(x, Wq, Wkv, Wproj):
    """Shard + lay out inputs for the 8 cores."""
    import ml_dtypes

    adt_np = ml_dtypes.bfloat16
    # multiplicative mask: keep where tk_local <= tq_local
    tri = np.where(
        np.arange(P)[:, None] <= np.arange(P)[None, :], 1.0, 0.0
    ).astype(np.float32)
    ident2 = np.concatenate([np.eye(64, dtype=np.float32)] * 2, axis=0).astype(
        adt_np
    )  # [128, 64]
    maskb = np.stack([tri, tri], axis=1).astype(adt_np)  # [128, 2, 128]
    vones = np.ones((P, TKC), dtype=adt_np)

    # per-TP-half weights (shared across the 4 batches)
    half_w = []
    for h in range(2):
        groups = [2 * h, 2 * h + 1]
        # pair p uses kv group p//2; order must match proj channel order below
        pairs = [(groups[0], groups[0] + 4), (groups[0] + 8, groups[0] + 12),
                 (groups[1], groups[1] + 4), (groups[1] + 8, groups[1] + 12)]
        cols = []
        for a, b in pairs:
            cols.append(Wq[a * D : (a + 1) * D, :].T)
            cols.append(Wq[b * D : (b + 1) * D, :].T)
        for g in groups:
            cols.append(Wkv[g * D : (g + 1) * D, :].T)  # k
            cols.append(Wkv[G * D + g * D : G * D + (g + 1) * D, :].T)  # v
        wqkv = np.concatenate(cols, axis=1).astype(adt_np)  # [1024, 768]
        wqkv4 = np.ascontiguousarray(wqkv.reshape(CC, P, 768).transpose(1, 0, 2))
        ch = np.concatenate(
            [np.arange(hh * D, (hh + 1) * D) for a, b in pairs for hh in (a, b)]
        )
        wproj_s = np.ascontiguousarray(Wproj[:, ch].T).astype(adt_np)  # [512,1024]
        wproj4 = np.ascontiguousarray(wproj_s.reshape(4, P, C).transpose(1, 0, 2))
        half_w.append((wqkv4, wproj4))

    in_maps = []
    for b in range(B):
        xT = np.ascontiguousarray(x[b].T).astype(adt_np)  # [1024, 2048]
        xt4 = np.ascontiguousarray(xT.reshape(CC, P, T))
        for h in range(2):
            wqkv4, wproj4 = half_w[h]
            in_maps.append(
                {
                    "xtl": xt4,
                    "wqkv": wqkv4,
                    "wproj": wproj4,
                    "maskb": maskb,
                    "ident2": ident2,
                    "vones": vones,
                }
            )
    return in_maps


def kernel(x, Wq, Wkv, Wproj, b_proj):
    x = np.asarray(x, dtype=np.float32)
    Wq = np.asarray(Wq, dtype=np.float32)
    Wkv = np.asarray(Wkv, dtype=np.float32)
    Wproj = np.asarray(Wproj, dtype=np.float32)
    b_proj = np.asarray(b_proj, dtype=np.float32)

    nc = _get_program()
    in_maps = _host_inputs(x, Wq, Wkv, Wproj)
    trace = bool(int(os.environ.get("BASS_KERNEL_TRACE", "0")))
    res = run_bass_kernel_spmd(nc, in_maps, list(range(8)), trace=trace)
    if trace:
        kernel.last_results = res

    out = np.empty((B, T, C), dtype=np.float32)
    for b in range(B):
        acc = res.results[2 * b]["outp"].astype(np.float32)
        acc = acc + res.results[2 * b + 1]["outp"].astype(np.float32)
        out[b] = acc + b_proj[None, :]
    return out
